# revision 18
# baseline (speedup 1.0000x reference)
"""Trainium2 Bass kernel for nn_ASRmodel_bg: batched 2D FFT convolution.

Reference math (per batch image, 1024x1024 complex grids):
    T = exp(i * z/c * Q);  field = H*W - ifft2(B)
    out = ifft2(fft2(field) * conj(T))

Since fft2 is linear: fft2(field) = fft2(H*W) - B, so
    out_n = ifft2(fft2(H_n * W) * conj(T)) - ifft2(B * conj(T))
         =  Fb @ ((F @ A_n @ F) o CS) @ Fb  -  D
where A_n = H_n o W (elementwise complex), F is the symmetric 1024-DFT
matrix, Fb = conj(F), CS = conj(T)/N^2 (ifft2 normalization folded in),
and D = Fb @ (B o CS) @ Fb is batch-shared.

On the PE, matmul computes lhsT.T @ rhs: feeding a stage's output back as
the next stage's lhsT transposes it for free, and with symmetric F the
whole chain needs zero explicit transposes. Inverse-DFT stages are made
forward-DFT stages by conjugate tracking (conj(X)^T F = conj(X^T Fb)),
so a single set of DFT matrices {Fr, Fi, Fr+Fi} serves every stage and
each complex matmul runs as 3 real matmuls (Karatsuba):
    M1 = Ar^T Fr, M2 = Ai^T Fi, M3 = (Ar+Ai)^T (Fr+Fi)
    real = M1 - M2, imag = M3 - M1 - M2.

Sharding: data-parallel over batch; 8 cores x 4 images. Each core computes
D redundantly. Matmuls run in float32r (TF32-like; ~1.2e-4 rel error per
1024-contraction at full bf16 speed).
"""

import numpy as np

C_LAMBDA = 6.37e-07
N_CORES = 8
N_BATCH = 32
MM = 1024  # grid size
P = 128

_cache = {}


def _build(mm, n_img, nf=None):
    """Build + compile the per-core Bass program (SPMD; all cores identical)."""
    import concourse.mybir as mybir
    import concourse.tile as tile
    from concourse import bacc

    f32 = mybir.dt.float32
    f32r = mybir.dt.float32r
    MULT = mybir.AluOpType.mult
    ADD = mybir.AluOpType.add
    SUB = mybir.AluOpType.subtract
    COPY = mybir.ActivationFunctionType.Copy

    KO = mm // P          # k-tiles (contraction blocks)
    MT = mm // P          # output m-tiles
    NF = nf or min(512, mm)  # psum free width (one bank)
    NH = mm // NF         # n-half count

    nc = bacc.Bacc("TRN2", target_bir_lowering=False, debug=False,
                   num_devices=N_CORES)

    h = nc.dram_tensor("h", [n_img, 2, mm, mm], f32, kind="ExternalInput").ap()
    w = nc.dram_tensor("w", [2, mm, mm], f32, kind="ExternalInput").ap()
    b = nc.dram_tensor("b", [2, mm, mm], f32, kind="ExternalInput").ap()
    c = nc.dram_tensor("c", [2, mm, mm], f32, kind="ExternalInput").ap()
    f = nc.dram_tensor("f", [3, mm, mm], f32, kind="ExternalInput").ap()
    o = nc.dram_tensor("o", [n_img, 2, mm, mm], f32, kind="ExternalOutput").ap()

    def colblock(ap2d):
        # [mm, cols] DRAM slice -> [P, KO, cols] (partition = row % 128)
        return ap2d.rearrange("(ko p) c -> p ko c", p=P)

    with tile.TileContext(nc) as tc:
        with (
            tc.tile_pool(name="const", bufs=1) as const,
            tc.tile_pool(name="region", bufs=1) as region,
            tc.tile_pool(name="lhs", bufs=2) as lhs,
            tc.tile_pool(name="raw", bufs=1) as raw,
            tc.tile_pool(name="cmt", bufs=1) as cmt,
            tc.tile_pool(name="dtmp", bufs=1) as dtmp,
            tc.tile_pool(name="psum", bufs=3, space="PSUM") as psum,
            tc.tile_pool(name="psum2", bufs=1, space="PSUM") as psum2,
            tc.tile_pool(name="dram", bufs=2, space="DRAM") as dram,
            tc.tile_pool(name="dramd", bufs=1, space="DRAM") as dramd,
        ):
            # --- DFT matrix planes (Fr, Fi, Fr+Fi), resident f32r ---
            fr_sb = const.tile([P, KO, mm], f32r, tag="fr", name="fr")
            fi_sb = const.tile([P, KO, mm], f32r, tag="fi", name="fi")
            fs_sb = const.tile([P, KO, mm], f32r, tag="fs", name="fs")
            nc.sync.dma_start(fr_sb[:], colblock(f[0].bitcast(f32r)))
            nc.sync.dma_start(fi_sb[:], colblock(f[1].bitcast(f32r)))
            nc.sync.dma_start(fs_sb[:], colblock(f[2].bitcast(f32r)))

            # D scratch: plane 0 = D_r, plane 1 = E := -D_i
            dsc_r = dramd.tile([mm, mm], f32, tag="dsc_r", name="dsc_r")
            dsc_e = dramd.tile([mm, mm], f32, tag="dsc_e", name="dsc_e")

            def new_region():
                return (region.tile([P, KO, mm], f32r, tag="reg_r", name="reg_r"),
                        region.tile([P, KO, mm], f32r, tag="reg_i", name="reg_i"))

            def build_cols(xr_ap, xi_ap, yr_ap, yi_ap, conj_out):
                """lhsT col-blocks (ar, ai, asum), f32r, of x*y or conj(x*y).

                x streams through `raw` (2 tags, double-buffered); y and the
                outputs ride the lhs pool's 2-slot round robin (y on even
                allocations, outputs on odd) so stream prefetch and matmul
                reads never collide.
                """
                xr = raw.tile([P, KO, P], f32, tag="s0", name="xr")
                nc.sync.dma_start(xr[:], colblock(xr_ap))
                xi = raw.tile([P, KO, P], f32, tag="s1", name="xi")
                nc.sync.dma_start(xi[:], colblock(xi_ap))
                yr = lhs.tile([P, KO, P], f32, tag="lhs_r", name="yr")
                nc.sync.dma_start(yr[:], colblock(yr_ap))
                yi = lhs.tile([P, KO, P], f32, tag="lhs_i", name="yi")
                nc.sync.dma_start(yi[:], colblock(yi_ap))

                ar = lhs.tile([P, KO, P], f32r, tag="lhs_r", name="ar")
                ai = lhs.tile([P, KO, P], f32r, tag="lhs_i", name="ai")
                asum = cmt.tile([P, KO, P], f32r, tag="lhs_s", name="asum")
                t0 = cmt.tile([P, KO, P], f32, tag="cm_t0", name="t0")
                t1 = cmt.tile([P, KO, P], f32, tag="cm_t1", name="t1")
                nc.vector.tensor_mul(t0[:], xr[:], yr[:])
                nc.gpsimd.tensor_mul(t1[:], xi[:], yi[:])
                t2 = cmt.tile([P, KO, P], f32, tag="cm_t0", name="t2")
                t3 = cmt.tile([P, KO, P], f32, tag="cm_t1", name="t3")
                if not conj_out:
                    # re = t0 - t1 ; im = t2 + t3
                    nc.vector.scalar_tensor_tensor(
                        ar[:], t1[:], -1.0, t0[:], op0=MULT, op1=ADD)
                    nc.gpsimd.tensor_mul(t2[:], xr[:], yi[:])
                    nc.vector.tensor_mul(t3[:], xi[:], yr[:])
                    nc.vector.tensor_add(ai[:], t2[:], t3[:])
                    nc.vector.tensor_add(asum[:], ar[:], ai[:])
                else:
                    # conj of (xr+ixi)(yr-iyi): re = t0 + t1 ; im = t2 - t3
                    # (x*conj(y): re = xr*yr + xi*yi, im = xi*yr - xr*yi;
                    #  conj flips im: im = xr*yi - xi*yr)
                    nc.vector.tensor_add(ar[:], t0[:], t1[:])
                    nc.gpsimd.tensor_mul(t2[:], xr[:], yi[:])
                    nc.vector.tensor_mul(t3[:], xi[:], yr[:])
                    nc.vector.scalar_tensor_tensor(
                        ai[:], t3[:], -1.0, t2[:], op0=MULT, op1=ADD)
                    nc.gpsimd.tensor_add(asum[:], ar[:], ai[:])
                return ar, ai, asum

            def region_cols(reg, m, eng_sum):
                rr, ri = reg
                cs = slice(m * P, (m + 1) * P)
                asum = lhs.tile([P, KO, P], f32r, tag="lhs_r", name="asum2")
                for kk in range(KO):
                    # per-k-slice adds: each depends on a single producer
                    # m-tile's drains, so consumer matmuls start while the
                    # producer stage is still running
                    eng_sum.tensor_add(asum[:, kk], rr[:, kk, cs], ri[:, kk, cs])
                return rr[:, :, cs], ri[:, :, cs], asum

            def cmm_stage(get_cols, drain, nh_outer=False):
                """Forward complex matmul stage via Karatsuba:
                M1 = ar^T Fr, M2 = ai^T Fi, M3 = asum^T Fs
                real = M1 - M2 ; imag = M3 - M1 - M2
                nh_outer=True drains a stage's low output columns first so a
                DRAM-bounced consumer can start before the stage finishes.
                """
                if nh_outer:
                    order = [(nh, m) for nh in range(NH) for m in range(MT)]
                else:
                    order = [(nh, m) for m in range(MT) for nh in range(NH)]
                cols = {}
                for nh, m in order:
                    if m not in cols or nh_outer:
                        cols[m] = get_cols(m, nh)
                    ar, ai, asum = cols[m]
                    if True:
                        m1 = psum.tile([P, NF], f32, tag="m1", name="m1")
                        m2 = psum2.tile([P, NF], f32, tag="m2", name="m2")
                        m3 = psum.tile([P, NF], f32, tag="m3", name="m3")
                        s = slice(nh * NF, (nh + 1) * NF)
                        for k in range(KO):
                            nc.tensor.matmul(m1[:], ar[:, k], fr_sb[:, k, s],
                                             start=(k == 0), stop=(k == KO - 1))
                        for k in range(KO):
                            nc.tensor.matmul(m2[:], ai[:, k], fi_sb[:, k, s],
                                             start=(k == 0), stop=(k == KO - 1))
                        for k in range(KO):
                            nc.tensor.matmul(m3[:], asum[:, k], fs_sb[:, k, s],
                                             start=(k == 0), stop=(k == KO - 1))
                        drain(m, nh, m1, m2, m3)

            def drain_re_im(m1, m2, m3, out_r, out_i):
                """out_r = M1-M2, out_i = M3-M1-M2 (= out_r - 2*M1 + M3)."""
                t = dtmp.tile([P, NF], f32, tag="ta", name="ta")
                nc.scalar.activation(t[:], m2[:], COPY, scale=-1.0)
                nc.vector.tensor_add(out_r, m1[:], t[:])
                q = dtmp.tile([P, NF], f32, tag="tb", name="tb")
                nc.vector.scalar_tensor_tensor(
                    q[:], m1[:], -2.0, out_r, op0=MULT, op1=ADD)
                nc.vector.tensor_add(out_i, q[:], m3[:])

            def drain_to_region(reg):
                rr, ri = reg

                def drain(m, nh, m1, m2, m3):
                    s = slice(nh * NF, (nh + 1) * NF)
                    drain_re_im(m1, m2, m3, rr[:, m, s], ri[:, m, s])

                return drain

            # ---------- D = conj(Fb (B o CS) Fb): store (D_r, -D_i) ----------
            # conj(B o CS) with CS=(c1,-c2): = (B conj y) pattern with y=(c1,c2)
            regA = new_region()

            def x_cols(m):
                cs = slice(m * P, (m + 1) * P)
                return build_cols(b[0][:, cs], b[1][:, cs],
                                  c[0][:, cs], c[1][:, cs], conj_out=True)

            cmm_stage(lambda m, nh: x_cols(m), drain=drain_to_region(regA))

            def d2_drain(m, nh, m1, m2, m3):
                rs = slice(m * P, (m + 1) * P)
                s = slice(nh * NF, (nh + 1) * NF)
                dr_o = dtmp.tile([P, NF], f32, tag="gr", name="do_r")
                de_o = dtmp.tile([P, NF], f32, tag="gi", name="do_e")
                drain_re_im(m1, m2, m3, dr_o[:], de_o[:])
                nc.sync.dma_start(dsc_r[rs, s], dr_o[:])
                nc.sync.dma_start(dsc_e[rs, s], de_o[:])

            cmm_stage(lambda m, nh: region_cols(regA, m, nc.vector),
                      drain=d2_drain)

            # ---------- images ----------
            for img in range(n_img):
                # stage 1: U = A^T F, A = H o W (true values)
                reg1 = new_region()

                def a_cols(m, img=img):
                    cs = slice(m * P, (m + 1) * P)
                    return build_cols(h[img, 0][:, cs], h[img, 1][:, cs],
                                      w[0][:, cs], w[1][:, cs], conj_out=False)

                cmm_stage(lambda m, nh: a_cols(m), drain=drain_to_region(reg1))

                # stage 2: G = U^T F (= fft2(A)); drain applies CS and writes
                # conj(M) = conj(G o CS) to DRAM
                msc_r = [dram.tile([mm, NF], f32, tag=f"msc_r{x}",
                                   name=f"msc_r{x}") for x in range(NH)]
                msc_i = [dram.tile([mm, NF], f32, tag=f"msc_i{x}",
                                   name=f"msc_i{x}") for x in range(NH)]

                def st2_drain(m, nh, m1, m2, m3):
                    rs = slice(m * P, (m + 1) * P)
                    s = slice(nh * NF, (nh + 1) * NF)
                    c1s = cmt.tile([P, NF], f32, tag="cm_t0", name="c1s")
                    c2s = cmt.tile([P, NF], f32, tag="cm_t1", name="c2s")
                    nc.sync.dma_start(c1s[:], c[0][rs, s])
                    nc.sync.dma_start(c2s[:], c[1][rs, s])
                    gr = dtmp.tile([P, NF], f32, tag="gr", name="gr")
                    gi = dtmp.tile([P, NF], f32, tag="gi", name="gi")
                    drain_re_im(m1, m2, m3, gr[:], gi[:])
                    # conj(M): Mr = gr*c1 + gi*c2 ; Mi' = gr*c2 - gi*c1
                    p1 = dtmp.tile([P, NF], f32, tag="ta", name="p1")
                    p2 = dtmp.tile([P, NF], f32, tag="tb", name="p2")
                    nc.vector.tensor_mul(p1[:], gr[:], c2s[:])
                    nc.gpsimd.tensor_mul(p2[:], gi[:], c1s[:])
                    # mi = p1 - p2, written in place over p1
                    nc.vector.scalar_tensor_tensor(
                        p1[:], p2[:], -1.0, p1[:], op0=MULT, op1=ADD)
                    nc.sync.dma_start(msc_i[nh][rs, :], p1[:])
                    # second pair, in place over gr/gi
                    nc.gpsimd.tensor_mul(gr[:], gr[:], c1s[:])
                    nc.vector.tensor_mul(gi[:], gi[:], c2s[:])
                    nc.vector.tensor_add(p2[:], gr[:], gi[:])
                    nc.sync.dma_start(msc_r[nh][rs, :], p2[:])

                cmm_stage(lambda m, nh, r=reg1: region_cols(r, m, nc.vector),
                          drain=st2_drain, nh_outer=True)

                # stage 3: conj(V) = conj(M)^T F
                reg3 = new_region()

                mph = MT // NH  # col-blocks per msc half

                def m_cols(m, nh, mr=msc_r, mi=msc_i):
                    half, lm = m // mph, m % mph
                    cs = slice(lm * P, (lm + 1) * P)
                    ar = lhs.tile([P, KO, P], f32r, tag="lhs_r", name="mar")
                    ai = lhs.tile([P, KO, P], f32r, tag="lhs_i", name="mai")
                    asum = lhs.tile([P, KO, P], f32r, tag="lhs_r", name="masum")
                    nc.sync.dma_start(
                        ar[:], colblock(mr[half][:, cs].bitcast(f32r)))
                    nc.sync.dma_start(
                        ai[:], colblock(mi[half][:, cs].bitcast(f32r)))
                    nc.gpsimd.tensor_add(asum[:], ar[:], ai[:])
                    return ar, ai, asum

                cmm_stage(m_cols, drain=drain_to_region(reg3))

                # stage 4: conj(OUTc) = conj(V)^T F; out = OUTc - D:
                #   out_r = (M1 - M2) - D_r
                #   out_i = M1 + M2 - M3 + E        (E = -D_i)
                def st4_drain(m, nh, m1, m2, m3, img=img):
                    rs = slice(m * P, (m + 1) * P)
                    s = slice(nh * NF, (nh + 1) * NF)
                    d1s = cmt.tile([P, NF], f32, tag="cm_t0", name="d1s")
                    d2s = cmt.tile([P, NF], f32, tag="cm_t1", name="d2s")
                    nc.sync.dma_start(d1s[:], dsc_r[rs, s])
                    nc.sync.dma_start(d2s[:], dsc_e[rs, s])
                    t = dtmp.tile([P, NF], f32, tag="ta", name="t4a")
                    u = dtmp.tile([P, NF], f32, tag="tb", name="t4b")
                    # out_r = (M1 - D_r) - M2
                    nc.scalar.activation(t[:], m2[:], COPY, scale=-1.0)
                    nc.vector.scalar_tensor_tensor(
                        u[:], d1s[:], -1.0, m1[:], op0=MULT, op1=ADD)
                    nc.gpsimd.tensor_add(u[:], u[:], t[:])
                    nc.sync.dma_start(o[img, 0][rs, s], u[:])
                    # out_i = ((M2 - M3) + M1) + E
                    t2 = dtmp.tile([P, NF], f32, tag="gr", name="t4c")
                    nc.scalar.activation(t2[:], m2[:], COPY)
                    q = dtmp.tile([P, NF], f32, tag="gi", name="t4d")
                    nc.vector.scalar_tensor_tensor(
                        q[:], m3[:], -1.0, t2[:], op0=MULT, op1=ADD)
                    nc.vector.tensor_add(q[:], q[:], m1[:])
                    nc.gpsimd.tensor_add(q[:], q[:], d2s[:])
                    nc.sync.dma_start(o[img, 1][rs, s], q[:])

                cmm_stage(lambda m, nh, r=reg3: region_cols(r, m, nc.vector),
                          drain=st4_drain)

    nc.compile()
    return nc


def _get_nc(mm=MM, n_img=N_BATCH // N_CORES):
    key = (mm, n_img)
    if key not in _cache:
        _cache[key] = _build(mm, n_img)
    return _cache[key]


def host_inputs(input, Q, z, W, B, mm=MM):
    """Host-side layout prep shared by all cores (no FFT math here)."""
    input = np.asarray(input, dtype=np.float32)
    Q = np.asarray(Q, dtype=np.float32)
    z = np.asarray(z, dtype=np.float32)
    W = np.asarray(W, dtype=np.float32)
    B = np.asarray(B, dtype=np.float32)

    h_planes = np.ascontiguousarray(np.moveaxis(input, -1, 1))  # [n, 2, mm, mm]
    w_planes = np.ascontiguousarray(np.moveaxis(W, -1, 0))      # [2, mm, mm]
    b_planes = np.ascontiguousarray(np.moveaxis(B, -1, 0))

    # CS = conj(T) / mm^2, T = exp(i * phase), phase = z/c * Q (fp32, like ref)
    phase = np.float32(z[0] / np.float32(C_LAMBDA)) * Q
    scale = np.float32(1.0 / (mm * mm))
    c_planes = np.stack([np.cos(phase) * scale, np.sin(phase) * scale]).astype(
        np.float32)

    # F planes: fr = cos(2pi jk/mm), fi = -sin(...), fs = fr + fi
    j = np.arange(mm, dtype=np.float64)
    jk = np.outer(j, j) % mm
    ang = 2.0 * np.pi / mm * jk
    fr = np.cos(ang)
    fi = -np.sin(ang)
    f_planes = np.stack([fr, fi, fr + fi]).astype(np.float32)

    return h_planes, w_planes, b_planes, c_planes, f_planes


def make_in_maps(input, Q, z, W, B, n_cores=N_CORES):
    h_planes, w_planes, b_planes, c_planes, f_planes = host_inputs(
        input, Q, z, W, B)
    n = h_planes.shape[0]
    per = n // n_cores
    maps = []
    for ci in range(n_cores):
        maps.append({
            "h": np.ascontiguousarray(h_planes[ci * per:(ci + 1) * per]),
            "w": w_planes,
            "b": b_planes,
            "c": c_planes,
            "f": f_planes,
        })
    return maps


def run_on_hw(input, Q, z, W, B, trace=False, tmpdir=None):
    from concourse.bass_utils import run_bass_kernel_spmd

    nc = _get_nc()
    in_maps = make_in_maps(input, Q, z, W, B)
    res = run_bass_kernel_spmd(
        nc, in_maps, core_ids=list(range(N_CORES)), trace=trace, tmpdir=tmpdir)
    outs = []
    for ci in range(N_CORES):
        op = res.results[ci]["o"]  # [per, 2, mm, mm]
        outs.append(np.moveaxis(op, 1, -1))  # [per, mm, mm, 2]
    full = np.concatenate(outs, axis=0).astype(np.float32)
    return full, res


def kernel(**inputs) -> np.ndarray:
    out, _ = run_on_hw(inputs["input"], inputs["Q"], inputs["z"],
                       inputs["W"], inputs["B"])
    return out


# revision 19
# speedup vs baseline: 1.0124x; 1.0124x over previous
"""Trainium2 Bass kernel for nn_ASRmodel_bg: batched 2D FFT convolution.

Reference math (per batch image, 1024x1024 complex grids):
    T = exp(i * z/c * Q);  field = H*W - ifft2(B)
    out = ifft2(fft2(field) * conj(T))

Since fft2 is linear, fft2(field) = fft2(H*W) - B, so per image
    out = Fb @ (((F @ A @ F) - B) o CS) @ Fb
with A = H o W (elementwise complex), F the symmetric 1024-point DFT
matrix, Fb = conj(F), and CS = conj(T)/N^2 (ifft2 normalization folded
into the frequency-domain mask). No batch-shared transform is needed at
all -- B is subtracted in the frequency domain inside stage 2's drain.

On the PE, matmul computes lhsT.T @ rhs: feeding a stage's output back as
the next stage's lhsT transposes it for free, and with symmetric F the
whole chain needs zero explicit transposes:
    stage1  U = A^T F          (fwd)
    stage2  G = U^T F = fft2(A);  M = (G - B) o CS   (drain-fused)
    stage3  V = M^T Fb         (inv)
    stage4  out = V^T Fb       (inv)
Complex products use 4 real matmuls accumulated in PSUM:
    fwd: pa = ArFr, pb = AiFi, pc = ArFi + AiFr -> re = pa-pb, im = pc
    inv: pa = ArFr + AiFi, pb = ArFi, pc = AiFr -> re = pa,    im = pc-pb

Sharding: data-parallel over batch; 8 cores x 4 images. Matmuls run in
float32r (TF32-like; ~1.5e-4 rel error per 1024-contraction at full
bf16-class speed).
"""

import numpy as np

C_LAMBDA = 6.37e-07
N_CORES = 8
N_BATCH = 32
MM = 1024  # grid size
P = 128

_cache = {}


def _build(mm, n_img, nf=None):
    """Build + compile the per-core Bass program (SPMD; all cores identical)."""
    import concourse.mybir as mybir
    import concourse.tile as tile
    from concourse import bacc

    f32 = mybir.dt.float32
    f32r = mybir.dt.float32r
    MULT = mybir.AluOpType.mult
    ADD = mybir.AluOpType.add
    SUB = mybir.AluOpType.subtract
    COPY = mybir.ActivationFunctionType.Copy

    KO = mm // P              # k-tiles (contraction blocks)
    MT = mm // P              # output m-tiles
    NF = nf or min(512, mm)   # psum free width (one bank)
    NH = mm // NF             # n-half count

    nc = bacc.Bacc("TRN2", target_bir_lowering=False, debug=False,
                   num_devices=N_CORES)

    h = nc.dram_tensor("h", [n_img, 2, mm, mm], f32, kind="ExternalInput").ap()
    w = nc.dram_tensor("w", [2, mm, mm], f32, kind="ExternalInput").ap()
    b = nc.dram_tensor("b", [2, mm, mm], f32, kind="ExternalInput").ap()
    c = nc.dram_tensor("c", [2, mm, mm], f32, kind="ExternalInput").ap()
    f = nc.dram_tensor("f", [2, mm, mm], f32, kind="ExternalInput").ap()
    o = nc.dram_tensor("o", [n_img, 2, mm, mm], f32, kind="ExternalOutput").ap()

    def colblock(ap2d):
        # [mm, cols] DRAM slice -> [P, KO, cols] (partition = row % 128)
        return ap2d.rearrange("(ko p) c -> p ko c", p=P)

    with tile.TileContext(nc) as tc:
        with (
            tc.tile_pool(name="const", bufs=1) as const,
            tc.tile_pool(name="region", bufs=1) as region,
            tc.tile_pool(name="lhs", bufs=2) as lhs,
            tc.tile_pool(name="raw", bufs=2) as raw,
            tc.tile_pool(name="cmt", bufs=1) as cmt,
            tc.tile_pool(name="dsl", bufs=2) as dsl,
            tc.tile_pool(name="dtmp", bufs=2) as dtmp,
            tc.tile_pool(name="psum", bufs=2, space="PSUM") as psum,
            tc.tile_pool(name="dram", bufs=2, space="DRAM") as dram,
        ):
            # --- DFT matrix planes (Fr, Fi), resident f32r ---
            fr_sb = const.tile([P, KO, mm], f32r, tag="fr", name="fr")
            fi_sb = const.tile([P, KO, mm], f32r, tag="fi", name="fi")
            nc.sync.dma_start(fr_sb[:], colblock(f[0].bitcast(f32r)))
            nc.sync.dma_start(fi_sb[:], colblock(f[1].bitcast(f32r)))

            def new_region():
                return (region.tile([P, KO, mm], f32r, tag="reg_r", name="reg_r"),
                        region.tile([P, KO, mm], f32r, tag="reg_i", name="reg_i"))

            def build_cols(xr_ap, xi_ap, yr_ap, yi_ap):
                """lhsT col-blocks (ar, ai) of the complex product x*y.

                x streams through `raw`; y and the outputs ride the lhs
                pool's 2-slot round robin (y even allocs, outputs odd) so
                stream prefetch and matmul reads never collide.
                """
                xr = raw.tile([P, KO, P], f32, tag="s0", name="xr")
                nc.sync.dma_start(xr[:], colblock(xr_ap))
                xi = raw.tile([P, KO, P], f32, tag="s1", name="xi")
                nc.sync.dma_start(xi[:], colblock(xi_ap))
                yr = lhs.tile([P, KO, P], f32, tag="lhs_r", name="yr")
                nc.sync.dma_start(yr[:], colblock(yr_ap))
                yi = lhs.tile([P, KO, P], f32, tag="lhs_i", name="yi")
                nc.sync.dma_start(yi[:], colblock(yi_ap))

                ar = lhs.tile([P, KO, P], f32r, tag="lhs_r", name="ar")
                ai = lhs.tile([P, KO, P], f32r, tag="lhs_i", name="ai")
                t0 = cmt.tile([P, KO, P], f32, tag="cm_t0", name="t0")
                t1 = cmt.tile([P, KO, P], f32, tag="cm_t1", name="t1")
                nc.vector.tensor_mul(t0[:], xr[:], yr[:])
                nc.gpsimd.tensor_mul(t1[:], xi[:], yi[:])
                nc.vector.scalar_tensor_tensor(
                    ar[:], t1[:], -1.0, t0[:], op0=MULT, op1=ADD)
                t2 = cmt.tile([P, KO, P], f32, tag="cm_t0", name="t2")
                t3 = cmt.tile([P, KO, P], f32, tag="cm_t1", name="t3")
                nc.gpsimd.tensor_mul(t2[:], xr[:], yi[:])
                nc.vector.tensor_mul(t3[:], xi[:], yr[:])
                nc.gpsimd.tensor_add(ai[:], t2[:], t3[:])
                return ar, ai

            def region_cols(reg, m):
                rr, ri = reg
                cs = slice(m * P, (m + 1) * P)
                return rr[:, :, cs], ri[:, :, cs]

            def cmm_stage(get_cols, drain, inv, nh_outer=False):
                """One complex matmul stage (see module docstring).

                nh_outer=True drains low output columns first so a
                DRAM-bounced consumer can start before the stage ends.
                """
                if nh_outer:
                    order = [(nh, m) for nh in range(NH) for m in range(MT)]
                else:
                    order = [(nh, m) for m in range(MT) for nh in range(NH)]
                cols = {}
                for nh, m in order:
                    if m not in cols or nh_outer:
                        cols[m] = get_cols(m)
                    ar, ai = cols[m]
                    pa = psum.tile([P, NF], f32, tag="pa", name="pa")
                    pb = psum.tile([P, NF], f32, tag="pb", name="pb")
                    pc = psum.tile([P, NF], f32, tag="pc", name="pc")
                    s = slice(nh * NF, (nh + 1) * NF)
                    if not inv:
                        for k in range(KO):
                            nc.tensor.matmul(pa[:], ar[:, k], fr_sb[:, k, s],
                                             start=(k == 0), stop=(k == KO - 1))
                        for k in range(KO):
                            nc.tensor.matmul(pb[:], ai[:, k], fi_sb[:, k, s],
                                             start=(k == 0), stop=(k == KO - 1))
                        for k in range(KO):
                            nc.tensor.matmul(pc[:], ar[:, k], fi_sb[:, k, s],
                                             start=(k == 0), stop=False)
                        for k in range(KO):
                            nc.tensor.matmul(pc[:], ai[:, k], fr_sb[:, k, s],
                                             start=False, stop=(k == KO - 1))
                    else:
                        for k in range(KO):
                            nc.tensor.matmul(pa[:], ar[:, k], fr_sb[:, k, s],
                                             start=(k == 0), stop=False)
                        for k in range(KO):
                            nc.tensor.matmul(pa[:], ai[:, k], fi_sb[:, k, s],
                                             start=False, stop=(k == KO - 1))
                        for k in range(KO):
                            nc.tensor.matmul(pb[:], ar[:, k], fi_sb[:, k, s],
                                             start=(k == 0), stop=(k == KO - 1))
                        for k in range(KO):
                            nc.tensor.matmul(pc[:], ai[:, k], fr_sb[:, k, s],
                                             start=(k == 0), stop=(k == KO - 1))
                    drain(m, nh, pa, pb, pc)

            def drain_fwd_region(reg):
                rr, ri = reg

                def drain(m, nh, pa, pb, pc):
                    s = slice(nh * NF, (nh + 1) * NF)
                    t = dtmp.tile([P, NF], f32, tag="ta", name="ta")
                    nc.scalar.activation(t[:], pb[:], COPY, scale=-1.0)
                    nc.vector.tensor_add(rr[:, m, s], pa[:], t[:])
                    nc.scalar.activation(ri[:, m, s], pc[:], COPY)

                return drain

            def drain_inv_region(reg):
                rr, ri = reg

                def drain(m, nh, pa, pb, pc):
                    s = slice(nh * NF, (nh + 1) * NF)
                    nc.scalar.activation(rr[:, m, s], pa[:], COPY)
                    t = dtmp.tile([P, NF], f32, tag="ta", name="ta")
                    nc.scalar.activation(t[:], pb[:], COPY, scale=-1.0)
                    nc.vector.tensor_add(ri[:, m, s], pc[:], t[:])

                return drain

            # ---------- images ----------
            for img in range(n_img):
                # stage 1: U = A^T F, A = H o W
                reg1 = new_region()

                def a_cols(m, img=img):
                    cs = slice(m * P, (m + 1) * P)
                    return build_cols(h[img, 0][:, cs], h[img, 1][:, cs],
                                      w[0][:, cs], w[1][:, cs])

                cmm_stage(a_cols, drain_fwd_region(reg1), inv=False)

                # stage 2: G = U^T F (= fft2(A));
                # drain computes M = (G - B) o CS, bounced to DRAM in halves
                msc_r = [dram.tile([mm, NF], f32, tag=f"msc_r{x}",
                                   name=f"msc_r{x}") for x in range(NH)]
                msc_i = [dram.tile([mm, NF], f32, tag=f"msc_i{x}",
                                   name=f"msc_i{x}") for x in range(NH)]

                def st2_drain(m, nh, pa, pb, pc):
                    rs = slice(m * P, (m + 1) * P)
                    s = slice(nh * NF, (nh + 1) * NF)
                    brs = dsl.tile([P, NF], f32, tag="b1", name="brs")
                    bis = dsl.tile([P, NF], f32, tag="b2", name="bis")
                    c1s = dsl.tile([P, NF], f32, tag="c1", name="c1s")
                    c2s = dsl.tile([P, NF], f32, tag="c2", name="c2s")
                    nc.sync.dma_start(brs[:], b[0][rs, s])
                    nc.sync.dma_start(bis[:], b[1][rs, s])
                    nc.sync.dma_start(c1s[:], c[0][rs, s])
                    nc.sync.dma_start(c2s[:], c[1][rs, s])
                    gr = dtmp.tile([P, NF], f32, tag="gr", name="gr")
                    gi = dtmp.tile([P, NF], f32, tag="gi", name="gi")
                    t = dtmp.tile([P, NF], f32, tag="ta", name="t2a")
                    # gr = (pa - pb) - Br ; gi = pc - Bi
                    nc.scalar.activation(t[:], pb[:], COPY, scale=-1.0)
                    nc.vector.tensor_add(gr[:], pa[:], t[:])
                    nc.gpsimd.tensor_sub(gr[:], gr[:], brs[:])
                    nc.vector.scalar_tensor_tensor(
                        gi[:], bis[:], -1.0, pc[:], op0=MULT, op1=ADD)
                    # M = (gr + i gi)(c1 - i c2):
                    #   Mi = gi*c1 - gr*c2 ; Mr = gr*c1 + gi*c2
                    p1 = dtmp.tile([P, NF], f32, tag="tb", name="p1")
                    p2 = dtmp.tile([P, NF], f32, tag="tc", name="p2")
                    nc.gpsimd.tensor_mul(p1[:], gr[:], c2s[:])
                    nc.vector.tensor_mul(p2[:], gi[:], c1s[:])
                    nc.vector.scalar_tensor_tensor(
                        p1[:], p1[:], -1.0, p2[:], op0=MULT, op1=ADD)
                    nc.sync.dma_start(msc_i[nh][rs, :], p1[:])
                    nc.vector.tensor_mul(gr[:], gr[:], c1s[:])
                    nc.gpsimd.tensor_mul(gi[:], gi[:], c2s[:])
                    nc.vector.tensor_add(p2[:], gr[:], gi[:])
                    nc.sync.dma_start(msc_r[nh][rs, :], p2[:])

                cmm_stage(lambda m, r=reg1: region_cols(r, m),
                          st2_drain, inv=False, nh_outer=True)

                # stage 3: V = M^T Fb
                reg3 = new_region()
                mph = MT // NH  # col-blocks per msc half

                def m_cols(m, mr=msc_r, mi=msc_i):
                    half, lm = m // mph, m % mph
                    cs = slice(lm * P, (lm + 1) * P)
                    ar = lhs.tile([P, KO, P], f32r, tag="lhs_r", name="mar")
                    ai = lhs.tile([P, KO, P], f32r, tag="lhs_i", name="mai")
                    nc.sync.dma_start(
                        ar[:], colblock(mr[half][:, cs].bitcast(f32r)))
                    nc.sync.dma_start(
                        ai[:], colblock(mi[half][:, cs].bitcast(f32r)))
                    return ar, ai

                cmm_stage(m_cols, drain_inv_region(reg3), inv=True)

                # stage 4: out = V^T Fb
                def st4_drain(m, nh, pa, pb, pc, img=img):
                    rs = slice(m * P, (m + 1) * P)
                    s = slice(nh * NF, (nh + 1) * NF)
                    or_o = dtmp.tile([P, NF], f32, tag="tb", name="or_o")
                    oi_o = dtmp.tile([P, NF], f32, tag="tc", name="oi_o")
                    nc.scalar.activation(or_o[:], pa[:], COPY)
                    nc.sync.dma_start(o[img, 0][rs, s], or_o[:])
                    t = dtmp.tile([P, NF], f32, tag="ta", name="t4a")
                    nc.scalar.activation(t[:], pb[:], COPY, scale=-1.0)
                    nc.vector.tensor_add(oi_o[:], pc[:], t[:])
                    nc.sync.dma_start(o[img, 1][rs, s], oi_o[:])

                cmm_stage(lambda m, r=reg3: region_cols(r, m),
                          st4_drain, inv=True)

    nc.compile()
    return nc


def _get_nc(mm=MM, n_img=N_BATCH // N_CORES):
    key = (mm, n_img)
    if key not in _cache:
        _cache[key] = _build(mm, n_img)
    return _cache[key]


def host_inputs(input, Q, z, W, B, mm=MM):
    """Host-side layout prep shared by all cores (no FFT math here)."""
    input = np.asarray(input, dtype=np.float32)
    Q = np.asarray(Q, dtype=np.float32)
    z = np.asarray(z, dtype=np.float32)
    W = np.asarray(W, dtype=np.float32)
    B = np.asarray(B, dtype=np.float32)

    h_planes = np.ascontiguousarray(np.moveaxis(input, -1, 1))  # [n, 2, mm, mm]
    w_planes = np.ascontiguousarray(np.moveaxis(W, -1, 0))      # [2, mm, mm]
    b_planes = np.ascontiguousarray(np.moveaxis(B, -1, 0))

    # CS = conj(T) / mm^2, T = exp(i * phase), phase = z/c * Q (fp32, like ref)
    phase = np.float32(z[0] / np.float32(C_LAMBDA)) * Q
    scale = np.float32(1.0 / (mm * mm))
    c_planes = np.stack([np.cos(phase) * scale, np.sin(phase) * scale]).astype(
        np.float32)

    # B is subtracted from fft2(A) before the CS mask; the mask absorbs the
    # 1/N^2, so B itself is passed unscaled.

    # F planes: fr = cos(2pi jk/mm), fi = -sin(...)
    j = np.arange(mm, dtype=np.float64)
    jk = np.outer(j, j) % mm
    ang = 2.0 * np.pi / mm * jk
    f_planes = np.stack([np.cos(ang), -np.sin(ang)]).astype(np.float32)

    return h_planes, w_planes, b_planes, c_planes, f_planes


def make_in_maps(input, Q, z, W, B, n_cores=N_CORES):
    h_planes, w_planes, b_planes, c_planes, f_planes = host_inputs(
        input, Q, z, W, B)
    n = h_planes.shape[0]
    per = n // n_cores
    maps = []
    for ci in range(n_cores):
        maps.append({
            "h": np.ascontiguousarray(h_planes[ci * per:(ci + 1) * per]),
            "w": w_planes,
            "b": b_planes,
            "c": c_planes,
            "f": f_planes,
        })
    return maps


def run_on_hw(input, Q, z, W, B, trace=False, tmpdir=None):
    from concourse.bass_utils import run_bass_kernel_spmd

    nc = _get_nc()
    in_maps = make_in_maps(input, Q, z, W, B)
    res = run_bass_kernel_spmd(
        nc, in_maps, core_ids=list(range(N_CORES)), trace=trace, tmpdir=tmpdir)
    outs = []
    for ci in range(N_CORES):
        op = res.results[ci]["o"]  # [per, 2, mm, mm]
        outs.append(np.moveaxis(op, 1, -1))  # [per, mm, mm, 2]
    full = np.concatenate(outs, axis=0).astype(np.float32)
    return full, res


def kernel(**inputs) -> np.ndarray:
    out, _ = run_on_hw(inputs["input"], inputs["Q"], inputs["z"],
                       inputs["W"], inputs["B"])
    return out


# revision 22
# speedup vs baseline: 1.2001x; 1.1854x over previous
"""Trainium2 Bass kernel for nn_ASRmodel_bg: batched 2D FFT convolution.

Reference math (per batch image, 1024x1024 complex grids):
    T = exp(i * z/c * Q);  field = H*W - ifft2(B)
    out = ifft2(fft2(field) * conj(T))

Since fft2 is linear: fft2(field) = fft2(H*W) - B, so
    out_n = ifft2(fft2(H_n * W) * conj(T)) - ifft2(B * conj(T))
         =  Fb @ ((F @ A_n @ F) o CS) @ Fb  -  D
where A_n = H_n o W (elementwise complex), F is the symmetric 1024-DFT
matrix, Fb = conj(F), CS = conj(T)/N^2 (ifft2 normalization folded in),
and D = Fb @ (B o CS) @ Fb is batch-shared.

On the PE, matmul computes lhsT.T @ rhs: feeding a stage's output back as
the next stage's lhsT transposes it for free, and with symmetric F the
whole chain needs zero explicit transposes. Inverse-DFT stages are made
forward-DFT stages by conjugate tracking (conj(X)^T F = conj(X^T Fb)),
so a single set of DFT matrices {Fr, Fi, Fr+Fi} serves every stage and
each complex matmul runs as 3 real matmuls (Karatsuba):
    M1 = Ar^T Fr, M2 = Ai^T Fi, M3 = (Ar+Ai)^T (Fr+Fi)
    real = M1 - M2, imag = M3 - M1 - M2.

Sharding: data-parallel over batch; 8 cores x 4 images. Each core computes
D redundantly. Matmuls run in float32r (TF32-like; ~1.2e-4 rel error per
1024-contraction at full bf16 speed).
"""

import numpy as np

C_LAMBDA = 6.37e-07
N_CORES = 8
N_BATCH = 32
MM = 1024  # grid size
P = 128

_cache = {}


def _build(mm, n_img, nf=None):
    """Build + compile the per-core Bass program (SPMD; all cores identical)."""
    import concourse.mybir as mybir
    import concourse.tile as tile
    from concourse import bacc

    f32 = mybir.dt.float32
    f32r = mybir.dt.float32r
    MULT = mybir.AluOpType.mult
    ADD = mybir.AluOpType.add
    SUB = mybir.AluOpType.subtract
    COPY = mybir.ActivationFunctionType.Copy

    KO = mm // P          # k-tiles (contraction blocks)
    MT = mm // P          # output m-tiles
    NF = nf or min(512, mm)  # psum free width (one bank)
    NH = mm // NF         # n-half count

    nc = bacc.Bacc("TRN2", target_bir_lowering=False, debug=False,
                   num_devices=N_CORES)

    h = nc.dram_tensor("h", [n_img, 2, mm, mm], f32, kind="ExternalInput").ap()
    w = nc.dram_tensor("w", [2, mm, mm], f32, kind="ExternalInput").ap()
    b = nc.dram_tensor("b", [2, mm, mm], f32, kind="ExternalInput").ap()
    c = nc.dram_tensor("c", [2, mm, mm], f32, kind="ExternalInput").ap()
    f = nc.dram_tensor("f", [3, mm, mm], f32, kind="ExternalInput").ap()
    o = nc.dram_tensor("o", [n_img, 2, mm, mm], f32, kind="ExternalOutput").ap()

    def colblock(ap2d):
        # [mm, cols] DRAM slice -> [P, KO, cols] (partition = row % 128)
        return ap2d.rearrange("(ko p) c -> p ko c", p=P)

    with tile.TileContext(nc) as tc:
        with (
            tc.tile_pool(name="const", bufs=1) as const,
            tc.tile_pool(name="region", bufs=1) as region,
            tc.tile_pool(name="lhs", bufs=2) as lhs,
            tc.tile_pool(name="raw", bufs=1) as raw,
            tc.tile_pool(name="cmt", bufs=1) as cmt,
            tc.tile_pool(name="dtmp", bufs=1) as dtmp,
            tc.tile_pool(name="psum", bufs=2, space="PSUM") as psum,
            tc.tile_pool(name="dram", bufs=2, space="DRAM") as dram,
        ):
            # --- DFT matrix planes (Fr, Fi, Fr+Fi), resident f32r ---
            fr_sb = const.tile([P, KO, mm], f32r, tag="fr", name="fr")
            fi_sb = const.tile([P, KO, mm], f32r, tag="fi", name="fi")
            fs_sb = const.tile([P, KO, mm], f32r, tag="fs", name="fs")
            nc.sync.dma_start(fr_sb[:], colblock(f[0].bitcast(f32r)))
            nc.sync.dma_start(fi_sb[:], colblock(f[1].bitcast(f32r)))
            nc.sync.dma_start(fs_sb[:], colblock(f[2].bitcast(f32r)))

            def new_region():
                return (region.tile([P, KO, mm], f32r, tag="reg_r", name="reg_r"),
                        region.tile([P, KO, mm], f32r, tag="reg_i", name="reg_i"))

            def build_cols(xr_ap, xi_ap, yr_ap, yi_ap, conj_out):
                """lhsT col-blocks (ar, ai, asum), f32r, of x*y or conj(x*y).

                x streams through `raw` (2 tags, double-buffered); y and the
                outputs ride the lhs pool's 2-slot round robin (y on even
                allocations, outputs on odd) so stream prefetch and matmul
                reads never collide.
                """
                xr = raw.tile([P, KO, P], f32, tag="s0", name="xr")
                nc.sync.dma_start(xr[:], colblock(xr_ap))
                xi = raw.tile([P, KO, P], f32, tag="s1", name="xi")
                nc.sync.dma_start(xi[:], colblock(xi_ap))
                yr = lhs.tile([P, KO, P], f32, tag="lhs_r", name="yr")
                nc.sync.dma_start(yr[:], colblock(yr_ap))
                yi = lhs.tile([P, KO, P], f32, tag="lhs_i", name="yi")
                nc.sync.dma_start(yi[:], colblock(yi_ap))

                ar = lhs.tile([P, KO, P], f32r, tag="lhs_r", name="ar")
                ai = lhs.tile([P, KO, P], f32r, tag="lhs_i", name="ai")
                asum = cmt.tile([P, KO, P], f32r, tag="lhs_s", name="asum")
                t0 = cmt.tile([P, KO, P], f32, tag="cm_t0", name="t0")
                t1 = cmt.tile([P, KO, P], f32, tag="cm_t1", name="t1")
                nc.vector.tensor_mul(t0[:], xr[:], yr[:])
                nc.gpsimd.tensor_mul(t1[:], xi[:], yi[:])
                t2 = cmt.tile([P, KO, P], f32, tag="cm_t0", name="t2")
                t3 = cmt.tile([P, KO, P], f32, tag="cm_t1", name="t3")
                if not conj_out:
                    # re = t0 - t1 ; im = t2 + t3
                    nc.vector.scalar_tensor_tensor(
                        ar[:], t1[:], -1.0, t0[:], op0=MULT, op1=ADD)
                    nc.gpsimd.tensor_mul(t2[:], xr[:], yi[:])
                    nc.vector.tensor_mul(t3[:], xi[:], yr[:])
                    nc.vector.tensor_add(ai[:], t2[:], t3[:])
                    nc.vector.tensor_add(asum[:], ar[:], ai[:])
                else:
                    # conj of (xr+ixi)(yr-iyi): re = t0 + t1 ; im = t2 - t3
                    # (x*conj(y): re = xr*yr + xi*yi, im = xi*yr - xr*yi;
                    #  conj flips im: im = xr*yi - xi*yr)
                    nc.vector.tensor_add(ar[:], t0[:], t1[:])
                    nc.gpsimd.tensor_mul(t2[:], xr[:], yi[:])
                    nc.vector.tensor_mul(t3[:], xi[:], yr[:])
                    nc.vector.scalar_tensor_tensor(
                        ai[:], t3[:], -1.0, t2[:], op0=MULT, op1=ADD)
                    nc.gpsimd.tensor_add(asum[:], ar[:], ai[:])
                return ar, ai, asum

            def region_cols(reg, m, eng_sum):
                rr, ri = reg
                cs = slice(m * P, (m + 1) * P)
                asum = lhs.tile([P, KO, P], f32r, tag="lhs_r", name="asum2")
                for kk in range(KO):
                    # per-k-slice adds: each depends on a single producer
                    # m-tile's drains, so consumer matmuls start while the
                    # producer stage is still running
                    eng_sum.tensor_add(asum[:, kk], rr[:, kk, cs], ri[:, kk, cs])
                return rr[:, :, cs], ri[:, :, cs], asum

            def cmm_stage(get_cols, drain, nh_outer=False):
                """Forward complex matmul stage via Karatsuba:
                M1 = ar^T Fr, M2 = ai^T Fi, M3 = asum^T Fs
                real = M1 - M2 ; imag = M3 - M1 - M2
                nh_outer=True drains a stage's low output columns first so a
                DRAM-bounced consumer can start before the stage finishes.
                """
                if nh_outer:
                    order = [(nh, m) for nh in range(NH) for m in range(MT)]
                else:
                    order = [(nh, m) for m in range(MT) for nh in range(NH)]
                cols = {}
                for nh, m in order:
                    if m not in cols or nh_outer:
                        cols[m] = get_cols(m, nh)
                    ar, ai, asum = cols[m]
                    if True:
                        m1 = psum.tile([P, NF], f32, tag="m1", name="m1")
                        m2 = psum.tile([P, NF], f32, tag="m2", name="m2")
                        m3 = psum.tile([P, NF], f32, tag="m3", name="m3")
                        s = slice(nh * NF, (nh + 1) * NF)
                        for k in range(KO):
                            nc.tensor.matmul(m1[:], ar[:, k], fr_sb[:, k, s],
                                             start=(k == 0), stop=(k == KO - 1))
                        for k in range(KO):
                            nc.tensor.matmul(m2[:], ai[:, k], fi_sb[:, k, s],
                                             start=(k == 0), stop=(k == KO - 1))
                        for k in range(KO):
                            nc.tensor.matmul(m3[:], asum[:, k], fs_sb[:, k, s],
                                             start=(k == 0), stop=(k == KO - 1))
                        drain(m, nh, m1, m2, m3)

            def drain_re_im(m1, m2, m3, out_r, out_i):
                """out_r = M1-M2, out_i = M3-M1-M2 (= out_r - 2*M1 + M3)."""
                t = dtmp.tile([P, NF], f32, tag="ta", name="ta")
                nc.scalar.activation(t[:], m2[:], COPY, scale=-1.0)
                nc.vector.tensor_add(out_r, m1[:], t[:])
                q = dtmp.tile([P, NF], f32, tag="tb", name="tb")
                nc.vector.scalar_tensor_tensor(
                    q[:], m1[:], -2.0, out_r, op0=MULT, op1=ADD)
                nc.vector.tensor_add(out_i, q[:], m3[:])

            def drain_to_region(reg):
                rr, ri = reg

                def drain(m, nh, m1, m2, m3):
                    s = slice(nh * NF, (nh + 1) * NF)
                    drain_re_im(m1, m2, m3, rr[:, m, s], ri[:, m, s])

                return drain

            # ---------- images ----------
            for img in range(n_img):
                # stage 1: U = A^T F, A = H o W (true values)
                reg1 = new_region()

                def a_cols(m, img=img):
                    cs = slice(m * P, (m + 1) * P)
                    return build_cols(h[img, 0][:, cs], h[img, 1][:, cs],
                                      w[0][:, cs], w[1][:, cs], conj_out=False)

                cmm_stage(lambda m, nh: a_cols(m), drain=drain_to_region(reg1))

                # stage 2: G = U^T F (= fft2(A)); drain applies CS and writes
                # conj(M) = conj(G o CS) to DRAM
                msc_r = [dram.tile([mm, NF], f32, tag=f"msc_r{x}",
                                   name=f"msc_r{x}") for x in range(NH)]
                msc_i = [dram.tile([mm, NF], f32, tag=f"msc_i{x}",
                                   name=f"msc_i{x}") for x in range(NH)]

                def st2_drain(m, nh, m1, m2, m3):
                    rs = slice(m * P, (m + 1) * P)
                    s = slice(nh * NF, (nh + 1) * NF)
                    c1s = cmt.tile([P, NF], f32, tag="cm_t0", name="c1s")
                    c2s = cmt.tile([P, NF], f32, tag="cm_t1", name="c2s")
                    nc.sync.dma_start(c1s[:], c[0][rs, s])
                    nc.sync.dma_start(c2s[:], c[1][rs, s])
                    brs = raw.tile([P, NF], f32, tag="s0", name="brs")
                    bis = raw.tile([P, NF], f32, tag="s1", name="bis")
                    nc.sync.dma_start(brs[:], b[0][rs, s])
                    nc.sync.dma_start(bis[:], b[1][rs, s])
                    gr = dtmp.tile([P, NF], f32, tag="gr", name="gr")
                    gi = dtmp.tile([P, NF], f32, tag="gi", name="gi")
                    drain_re_im(m1, m2, m3, gr[:], gi[:])
                    nc.gpsimd.tensor_sub(gr[:], gr[:], brs[:])
                    nc.gpsimd.tensor_sub(gi[:], gi[:], bis[:])
                    # conj(M): Mr = gr*c1 + gi*c2 ; Mi' = gr*c2 - gi*c1
                    p1 = dtmp.tile([P, NF], f32, tag="ta", name="p1")
                    p2 = dtmp.tile([P, NF], f32, tag="tb", name="p2")
                    nc.vector.tensor_mul(p1[:], gr[:], c2s[:])
                    nc.gpsimd.tensor_mul(p2[:], gi[:], c1s[:])
                    # mi = p1 - p2, written in place over p1
                    nc.vector.scalar_tensor_tensor(
                        p1[:], p2[:], -1.0, p1[:], op0=MULT, op1=ADD)
                    nc.sync.dma_start(msc_i[nh][rs, :], p1[:])
                    # second pair, in place over gr/gi
                    nc.gpsimd.tensor_mul(gr[:], gr[:], c1s[:])
                    nc.vector.tensor_mul(gi[:], gi[:], c2s[:])
                    nc.vector.tensor_add(p2[:], gr[:], gi[:])
                    nc.sync.dma_start(msc_r[nh][rs, :], p2[:])

                cmm_stage(lambda m, nh, r=reg1: region_cols(r, m, nc.vector),
                          drain=st2_drain, nh_outer=True)

                # stage 3: conj(V) = conj(M)^T F
                reg3 = new_region()

                mph = MT // NH  # col-blocks per msc half

                def m_cols(m, nh, mr=msc_r, mi=msc_i):
                    half, lm = m // mph, m % mph
                    cs = slice(lm * P, (lm + 1) * P)
                    ar = lhs.tile([P, KO, P], f32r, tag="lhs_r", name="mar")
                    ai = lhs.tile([P, KO, P], f32r, tag="lhs_i", name="mai")
                    asum = lhs.tile([P, KO, P], f32r, tag="lhs_r", name="masum")
                    nc.sync.dma_start(
                        ar[:], colblock(mr[half][:, cs].bitcast(f32r)))
                    nc.sync.dma_start(
                        ai[:], colblock(mi[half][:, cs].bitcast(f32r)))
                    nc.gpsimd.tensor_add(asum[:], ar[:], ai[:])
                    return ar, ai, asum

                cmm_stage(m_cols, drain=drain_to_region(reg3))

                # stage 4: conj(OUTc) = conj(V)^T F; out = OUTc - D:
                #   out_r = (M1 - M2) - D_r
                #   out_i = M1 + M2 - M3 + E        (E = -D_i)
                def st4_drain(m, nh, m1, m2, m3, img=img):
                    rs = slice(m * P, (m + 1) * P)
                    s = slice(nh * NF, (nh + 1) * NF)
                    t = dtmp.tile([P, NF], f32, tag="ta", name="t4a")
                    u = dtmp.tile([P, NF], f32, tag="tb", name="t4b")
                    # out_r = M1 - M2
                    nc.scalar.activation(t[:], m2[:], COPY, scale=-1.0)
                    nc.vector.tensor_add(u[:], m1[:], t[:])
                    nc.sync.dma_start(o[img, 0][rs, s], u[:])
                    # out_i = (M2 - M3) + M1
                    t2 = dtmp.tile([P, NF], f32, tag="gr", name="t4c")
                    nc.scalar.activation(t2[:], m2[:], COPY)
                    q = dtmp.tile([P, NF], f32, tag="gi", name="t4d")
                    nc.vector.scalar_tensor_tensor(
                        q[:], m3[:], -1.0, t2[:], op0=MULT, op1=ADD)
                    nc.vector.tensor_add(q[:], q[:], m1[:])
                    nc.sync.dma_start(o[img, 1][rs, s], q[:])

                cmm_stage(lambda m, nh, r=reg3: region_cols(r, m, nc.vector),
                          drain=st4_drain)

    nc.compile()
    return nc


def _get_nc(mm=MM, n_img=N_BATCH // N_CORES):
    key = (mm, n_img)
    if key not in _cache:
        _cache[key] = _build(mm, n_img)
    return _cache[key]


def host_inputs(input, Q, z, W, B, mm=MM):
    """Host-side layout prep shared by all cores (no FFT math here)."""
    input = np.asarray(input, dtype=np.float32)
    Q = np.asarray(Q, dtype=np.float32)
    z = np.asarray(z, dtype=np.float32)
    W = np.asarray(W, dtype=np.float32)
    B = np.asarray(B, dtype=np.float32)

    h_planes = np.ascontiguousarray(np.moveaxis(input, -1, 1))  # [n, 2, mm, mm]
    w_planes = np.ascontiguousarray(np.moveaxis(W, -1, 0))      # [2, mm, mm]
    b_planes = np.ascontiguousarray(np.moveaxis(B, -1, 0))

    # CS = conj(T) / mm^2, T = exp(i * phase), phase = z/c * Q (fp32, like ref)
    phase = np.float32(z[0] / np.float32(C_LAMBDA)) * Q
    scale = np.float32(1.0 / (mm * mm))
    c_planes = np.stack([np.cos(phase) * scale, np.sin(phase) * scale]).astype(
        np.float32)

    # F planes: fr = cos(2pi jk/mm), fi = -sin(...), fs = fr + fi
    j = np.arange(mm, dtype=np.float64)
    jk = np.outer(j, j) % mm
    ang = 2.0 * np.pi / mm * jk
    fr = np.cos(ang)
    fi = -np.sin(ang)
    f_planes = np.stack([fr, fi, fr + fi]).astype(np.float32)

    return h_planes, w_planes, b_planes, c_planes, f_planes


def make_in_maps(input, Q, z, W, B, n_cores=N_CORES):
    h_planes, w_planes, b_planes, c_planes, f_planes = host_inputs(
        input, Q, z, W, B)
    n = h_planes.shape[0]
    per = n // n_cores
    maps = []
    for ci in range(n_cores):
        maps.append({
            "h": np.ascontiguousarray(h_planes[ci * per:(ci + 1) * per]),
            "w": w_planes,
            "b": b_planes,
            "c": c_planes,
            "f": f_planes,
        })
    return maps


def run_on_hw(input, Q, z, W, B, trace=False, tmpdir=None):
    from concourse.bass_utils import run_bass_kernel_spmd

    nc = _get_nc()
    in_maps = make_in_maps(input, Q, z, W, B)
    res = run_bass_kernel_spmd(
        nc, in_maps, core_ids=list(range(N_CORES)), trace=trace, tmpdir=tmpdir)
    outs = []
    for ci in range(N_CORES):
        op = res.results[ci]["o"]  # [per, 2, mm, mm]
        outs.append(np.moveaxis(op, 1, -1))  # [per, mm, mm, 2]
    full = np.concatenate(outs, axis=0).astype(np.float32)
    return full, res


def kernel(**inputs) -> np.ndarray:
    out, _ = run_on_hw(inputs["input"], inputs["Q"], inputs["z"],
                       inputs["W"], inputs["B"])
    return out
